# revision 1
# baseline (speedup 1.0000x reference)
"""BiLSTM (B=16, T=2048, D=U=256) on 8 TRN2 NeuronCores.

Sharding: 8 cores = 2 directions x 4 batch-shards (B_local=4 per core).
Backward cores receive x time-reversed on the host; all cores run the same
SPMD program (a forward scan), so no collectives are needed.  Keras-style
go_backwards semantics mean the backward half is emitted in iteration
order, which is exactly the scan order on the backward cores.

Per-core kernel: precompute xw[t] = x_t @ W on the TensorEngine (bf16,
gate order [cand i f o], candidate columns pre-doubled on the host so
tanh(x) = 2*sigmoid(2x)-1 needs only the sigmoid table), then run the
2048-step recurrence.  Per step the PE accumulates 16 R-tile matmuls
(R and h in float8e4 — halves the per-step LDWEIGHTS stream; validated
rel err ~5e-3) on top of identity-matmuls that inject xw_t into PSUM,
with the [cand,i,f] and [o] gates in separate PSUM banks so ScalarE's
batched sigmoid over [cand|i|f] can start while the o-chunk matmuls
finish.  VectorE does the cell update (cand affine, one fused
[i|f]*[cand|c] multiply, pair add); tanh(sigmoid(s)) is approximated as
K*sigmoid(AL*s + BE) (max err 8.6e-4) so the output nonlinearity is a
single ScalarE op with K folded into R and the output copy, and
c' = sigmoid(s) runs off the critical path.  h is written twice: fp8 for
the recurrence, bf16 for the staged f32 output.
"""

import numpy as np

F32 = None  # set on first build

_CACHE = {}

T = 2048
B = 16
D = 256
U = 256
G = 4 * U
BL = 4  # batch per core

K_PHI = 0.7589144336406901
AL_PHI = 1.0834263081088795
BE_PHI = 0.44379053813456204


def _patch_tile_drain():
    """This container's walrus accepts only one sem-wait/update per
    instruction; spread Tile's final-drain waits across NOPs."""
    import concourse.tile as tile
    import concourse.mybir as mybir
    from concourse.vector_clock import ScopedClock

    if getattr(tile.TileContext, "_lstm_patched", False):
        return

    def _drain_and_barrier(self, tick_clock, wait_clock):
        carrier = self.nc.sync.nop(nofuse=True, hint="final_wait_carrier")
        wait_clock.add_sem_waits(
            carrier.ins, ScopedClock({None: tick_clock.global_clock})
        )
        si = carrier.ins.sync_info
        waits = list(si.on_wait or []) if si is not None else []
        if len(waits) > 1:
            si.on_wait = waits[:1]
            for wx in waits[1:]:
                n = self.nc.sync.nop(nofuse=True, hint="final_wait_extra")
                if n.ins.sync_info is None:
                    n.ins.sync_info = mybir.SyncInfo(on_wait=[wx], on_update=[])
                else:
                    n.ins.sync_info.on_wait = [wx]
        self.nc.sync.drain()
        self.nc.all_engine_barrier()
        assert self.sems is not None
        popped = self.nc._tile_sem_poison_stack.pop()
        assert popped is self._sem_poison
        self.nc.clear_and_free_semaphores(list(self.sems.allocated().values()))
        self.nc.all_engine_barrier()

    tile.TileContext._drain_and_barrier = _drain_and_barrier
    tile.TileContext._lstm_patched = True


def _split_syncs(nc, max_waits=1, max_updates=1):
    import concourse.mybir as mybir

    ctr = [0]

    def mknop(engine, waits, updates):
        ctr[0] += 1
        return mybir.InstNoOp(
            name=f"syncfix-{ctr[0]}",
            engine=engine,
            sync_info=mybir.SyncInfo(on_wait=list(waits), on_update=list(updates)),
        )

    for f in nc.m.functions:
        for bb in f.blocks:
            changed = False
            out = []
            for inst in bb.instructions:
                si = inst.sync_info
                if si is None or inst.engine == mybir.EngineType.Unassigned:
                    out.append(inst)
                    continue
                waits = list(si.on_wait or [])
                updates = list(si.on_update or [])
                if len(waits) <= max_waits and len(updates) <= max_updates:
                    out.append(inst)
                    continue
                changed = True
                for wx in waits[:-max_waits] if max_waits else waits:
                    out.append(mknop(inst.engine, [wx], []))
                si.on_wait = waits[-max_waits:] if max_waits else []
                extra_u = updates[max_updates:] if max_updates else updates
                si.on_update = updates[:max_updates] if max_updates else []
                out.append(inst)
                for ux in extra_u:
                    out.append(mknop(inst.engine, [], [ux]))
            if changed:
                bb.instructions = out
    return nc


def _build_v3(seg=128, proj_tb=128, split_sig=True, fp8=True, use_tanh=False,
              hfull=True, B=BL):
    import concourse.bass as bass
    import concourse.mybir as mybir
    import concourse.tile as tile
    from contextlib import ExitStack

    _patch_tile_drain()
    F32 = mybir.dt.float32
    BF16 = mybir.dt.bfloat16
    FP8 = mybir.dt.float8e4
    SIG = mybir.ActivationFunctionType.Sigmoid
    COPY = mybir.ActivationFunctionType.Copy
    nc = bass.Bass()
    xt = nc.dram_tensor("xt", [2, 128, T * B], F32, kind="ExternalInput")
    w = nc.dram_tensor("w", [D, G], F32, kind="ExternalInput")
    r = nc.dram_tensor("r", [U, G], F32, kind="ExternalInput")
    bcg = nc.dram_tensor("bcg", [128, 2], F32, kind="ExternalInput")
    out = nc.dram_tensor("out", [2, 128, T * B], F32, kind="ExternalOutput")

    RDT = FP8 if fp8 else BF16
    NB = B
    HW = 2 * NB
    W8 = 8 * NB

    with ExitStack() as ctx:
        tc = ctx.enter_context(tile.TileContext(nc))
        const = ctx.enter_context(tc.tile_pool(name="const", bufs=1))
        big = ctx.enter_context(tc.tile_pool(name="big", bufs=1))
        wstage = ctx.enter_context(tc.tile_pool(name="wstage", bufs=2))
        xload = ctx.enter_context(tc.tile_pool(name="xload", bufs=2))
        xcast = ctx.enter_context(tc.tile_pool(name="xcast", bufs=2))
        ppsum = ctx.enter_context(tc.tile_pool(name="ppsum", bufs=2, space="PSUM"))
        gpsum = ctx.enter_context(tc.tile_pool(name="gpsum", bufs=2, space="PSUM"))
        work = ctx.enter_context(tc.tile_pool(name="work", bufs=3))
        hsegp = ctx.enter_context(tc.tile_pool(name="hsegp", bufs=2))
        ostage = ctx.enter_context(tc.tile_pool(name="ostage", bufs=2))

        wb = const.tile([128, 2, G], BF16)
        rb = const.tile([128, 2, G], RDT)
        bct = const.tile([128, 2], F32)
        ident = const.tile([128, 128], BF16)
        hzero = const.tile([128, HW], RDT)
        bphi = const.tile([128, 1], F32)
        nc.vector.memset(bphi[:, :], BE_PHI)

        for src, dst in ((w, wb), (r, rb)):
            for k in range(2):
                st = wstage.tile([128, G], F32, tag="wst")
                nc.sync.dma_start(out=st[:, :], in_=src[k * 128:(k + 1) * 128, :])
                nc.scalar.copy(dst[:, k, :], st[:, :])
        nc.sync.dma_start(out=bct[:, :], in_=bcg[:, :])
        from concourse.masks import make_identity
        make_identity(nc, ident[:, :])
        nc.vector.memset(hzero[:, :], 0.0)

        xw = big.tile([128, T, W8], BF16)
        if hfull:
            h2 = big.tile([128, T, HW], RDT)
        else:
            h2 = big.tile([128, 2, HW], RDT)  # fp8 h/K ping-pong

        ntb = T // proj_tb
        ntok = proj_tb * B
        for tb in range(ntb):
            t0 = tb * proj_tb
            xf = xload.tile([128, 2, ntok], F32)
            xb = xcast.tile([128, 2, ntok], BF16)
            for k in range(2):
                nc.sync.dma_start(
                    out=xf[:, k, :], in_=xt[k, :, t0 * B:(t0 + proj_tb) * B],
                )
            nc.scalar.copy(xb[:, :, :], xf[:, :, :])
            for c in range(8):
                ps = ppsum.tile([128, ntok], F32)
                for k in range(2):
                    nc.tensor.matmul(
                        ps[:, :],
                        wb[:, k, c * 128:(c + 1) * 128],
                        xb[:, k, :],
                        start=(k == 0),
                        stop=(k == 1),
                    )
                dst = xw[:, t0:t0 + proj_tb, c * NB:(c + 1) * NB]
                if c < 2:  # cand chunks carry the bias
                    nc.vector.tensor_scalar(
                        dst, ps[:, :], bct[:, c:c + 1], None,
                        mybir.AluOpType.add,
                    )
                elif c % 2 == 0:
                    nc.scalar.copy(dst, ps[:, :])
                else:
                    nc.vector.tensor_copy(dst, ps[:, :])

        # state: cand (0:HW) | c (HW:2HW)
        state = const.tile([128, 2 * HW], F32)
        nc.vector.memset(state[:, :], 0.0)
        nseg = T // seg
        for si in range(nseg):
            hseg = hsegp.tile([128, seg, HW], BF16)
            for tl in range(seg):
                t = si * seg + tl
                g = gpsum.tile([128, 6 * NB], F32, tag="gcif")
                go = gpsum.tile([128, 2 * NB], F32, tag="go")
                nc.tensor.matmul(
                    g[:, :], ident[:, :], xw[:, t, :6 * NB],
                    start=True, stop=False, skip_group_check=True,
                )
                nc.tensor.matmul(
                    go[:, :], ident[:, :], xw[:, t, 6 * NB:],
                    start=True, stop=False, skip_group_check=True,
                )

                def rmm(c, k, stop=False):
                    rhs = (hzero[:, k * NB:(k + 1) * NB] if t == 0
                           else h2[:, (t - 1) if hfull else (t - 1) % 2, k * NB:(k + 1) * NB])
                    dst = (g[:, c * NB:(c + 1) * NB] if c < 6
                           else go[:, (c - 6) * NB:(c - 5) * NB])
                    nc.tensor.matmul(
                        dst,
                        rb[:, k, c * 128:(c + 1) * 128],
                        rhs,
                        start=False, stop=stop, skip_group_check=True,
                    )

                u = work.tile([128, W8], F32, tag="u")
                for c in range(6):
                    for k in range(2):
                        rmm(c, k)
                TANH = mybir.ActivationFunctionType.Tanh
                if split_sig:
                    if use_tanh:
                        # cand = tanh(g_c) straight into state[:, 0:HW]
                        nc.scalar.activation(state[:, :HW], g[:, :2 * NB],
                                             TANH, scale=0.5)
                        nc.scalar.activation(u[:, HW:6 * NB], g[:, 2 * NB:], SIG)
                    else:
                        nc.scalar.activation(u[:, :6 * NB], g[:, :], SIG)
                for c in (6, 7):
                    for k in range(2):
                        rmm(c, k, stop=(c == 7 and k == 1))
                if split_sig:
                    nc.scalar.activation(u[:, 6 * NB:], go[:, :], SIG)
                else:
                    nc.scalar.activation(u[:, :6 * NB], g[:, :], SIG)
                    nc.scalar.activation(u[:, 6 * NB:], go[:, :], SIG)
                if not use_tanh:
                    # cand = 2*u_c - 1 -> state[:, 0:HW]
                    nc.vector.tensor_scalar(
                        state[:, :HW], u[:, :HW], 2.0, -1.0,
                        mybir.AluOpType.mult, mybir.AluOpType.add,
                    )
                # prod = [i|f] * [cand|c] -> [m2|m1]
                prod = work.tile([128, 4 * HW], F32, tag="prod")
                nc.vector.tensor_mul(
                    prod[:, :2 * HW], u[:, HW:3 * HW], state[:, :],
                )
                s = prod[:, 2 * HW:3 * HW]
                nc.vector.tensor_add(s, prod[:, :HW], prod[:, HW:2 * HW])
                phi = prod[:, 3 * HW:]
                nc.scalar.activation(phi, s, SIG, bias=bphi[:, :], scale=AL_PHI)
                nc.vector.tensor_mul(
                    h2[:, t if hfull else t % 2, :], phi, u[:, 6 * NB:],
                )
                nc.scalar.activation(state[:, HW:], s, SIG)
                nc.vector.tensor_mul(
                    hseg[:, tl, :], phi, u[:, 6 * NB:],
                )

            t0 = si * seg
            ost = ostage.tile([128, 2, seg, NB], F32)
            nc.scalar.activation(
                ost[:, :, :, :],
                hseg[:, :, :].rearrange("p t (k b) -> p k t b", k=2),
                COPY, scale=K_PHI,
            )
            for k in range(2):
                nc.sync.dma_start(
                    out=out[k, :, t0 * B:(t0 + seg) * B],
                    in_=ost[:, k, :, :],
                )
    _split_syncs(nc)
    return nc


def _prep_weights(Wd, Rd, bcd):
    # reference gate order [i f o c] -> kernel order [c i f o]
    perm = np.concatenate([
        np.arange(3 * U, 4 * U), np.arange(0, U),
        np.arange(U, 2 * U), np.arange(2 * U, 3 * U),
    ])
    Wp = np.ascontiguousarray(Wd[:, perm]).astype(np.float32)
    Rp = np.ascontiguousarray(Rd[:, perm]).astype(np.float32)
    Wp[:, :U] *= 2.0
    Rp[:, :U] *= 2.0
    Rp *= K_PHI
    bcg = np.ascontiguousarray((2.0 * bcd).reshape(2, 128).T).astype(np.float32)
    return Wp, Rp, bcg


def kernel(x, W_f, R_f, bc_f, W_b, R_b, bc_b):
    from concourse.bass_utils import run_bass_kernel_spmd

    x = np.asarray(x, dtype=np.float32)
    if "nc" not in _CACHE:
        _CACHE["nc"] = _build_v3()
    nc = _CACHE["nc"]

    Wf, Rf, bcgf = _prep_weights(np.asarray(W_f, np.float32),
                                 np.asarray(R_f, np.float32),
                                 np.asarray(bc_f, np.float32))
    Wb_, Rb_, bcgb = _prep_weights(np.asarray(W_b, np.float32),
                                   np.asarray(R_b, np.float32),
                                   np.asarray(bc_b, np.float32))

    in_maps = []
    for core in range(8):
        fwd = core < 4
        b0 = (core % 4) * BL
        xs = x[b0:b0 + BL]
        if not fwd:
            xs = xs[:, ::-1, :]
        xtr = np.ascontiguousarray(xs.transpose(2, 1, 0)).reshape(2, 128, T * BL)
        in_maps.append({
            "xt": xtr,
            "w": Wf if fwd else Wb_,
            "r": Rf if fwd else Rb_,
            "bcg": bcgf if fwd else bcgb,
        })

    res = run_bass_kernel_spmd(nc, in_maps, core_ids=list(range(8)))

    outp = np.empty((B, T, 2 * U), dtype=np.float32)
    for core in range(8):
        od = res.results[core]["out"]  # [2, 128, T*BL]
        hb = od.reshape(256, T, BL).transpose(2, 1, 0)  # [BL, T, U]
        b0 = (core % 4) * BL
        if core < 4:
            outp[b0:b0 + BL, :, 0:U] = hb
        else:
            outp[b0:b0 + BL, :, U:2 * U] = hb
    return outp



# revision 9
# speedup vs baseline: 10.5036x; 10.5036x over previous
"""BiLSTM (B=16, T=2048, D=U=256) on 8 TRN2 NeuronCores.

Sharding: 8 cores = 2 directions x 4 batch-shards (B_local=4 per core).
Backward cores receive x time-reversed on the host; all cores run the same
SPMD program (a forward scan), so no collectives are needed.

Chunked-parallel scan: the cell update c' = sigmoid(f*c + i*cand) is
strongly contracting (|dc'/dc| <= f/4), so state influence decays ~7x per
step.  T is therefore split into KM chunks, each warmed up from zero state
over the P steps preceding its range (error ~1e-6 at P=8).  The chunks run
as M bundles of K lanes: the K lanes of a bundle advance in lockstep, so
every engine instruction covers all K lanes at once (amortizing per-
instruction fixed costs), while the M bundles interleave in program order
to hide the ~2us per-step cross-engine latency chain.

Per fused step: PE accumulates x_t@W (lanes strided across chunks, zero-
padded left edge covers warmup before t=0), a rank-1 bias injection for
the candidate gate, and h@R into one PSUM tile; ScalarE applies one
sigmoid over all 4*K gate groups (candidate columns pre-doubled so
tanh(a) = 2*sigmoid(2a)-1); DVE computes the cell update in 4 fused ops;
a second ScalarE sigmoid yields [c' | phi] jointly where
tanh(sigmoid(s)) ~= K_PHI*sigmoid(AL*s+BE); one last DVE op writes
h = K_PHI*phi*o straight into the bf16 h-history that doubles as the
DMA-out staging buffer (output dram is bf16; host converts to f32).
"""

import numpy as np

_CACHE = {}

T = 2048
D = 256
U = 256
G = 4 * U
BL = 4  # batch per core

KL = 8   # lanes (chunks) per bundle, fused per instruction
MB = 3   # bundles, staggered to hide latency
PW = 8   # warmup steps per chunk
KM = KL * MB
TC = -(-T // KM)          # chunk length (T padded up to KM*TC)
TP = KM * TC
NS = TC + PW              # steps per chain
TS = NS + 1               # h slots (slot 0 = initial zeros)
KB = KL * BL              # fused free width per gate chunk

K_PHI = 0.7589144336406901
AL_PHI = 1.0834263081088795
BE_PHI = 0.44379053813456204


def _patch_tile_drain():
    """This container's walrus accepts only one sem-wait/update per
    instruction; spread Tile's final-drain waits across NOPs."""
    import concourse.tile as tile
    import concourse.mybir as mybir
    from concourse.vector_clock import ScopedClock

    if getattr(tile.TileContext, "_lstm_patched", False):
        return

    def _drain_and_barrier(self, tick_clock, wait_clock):
        carrier = self.nc.sync.nop(nofuse=True, hint="final_wait_carrier")
        wait_clock.add_sem_waits(
            carrier.ins, ScopedClock({None: tick_clock.global_clock})
        )
        si = carrier.ins.sync_info
        waits = list(si.on_wait or []) if si is not None else []
        if len(waits) > 1:
            si.on_wait = waits[:1]
            for wx in waits[1:]:
                n = self.nc.sync.nop(nofuse=True, hint="final_wait_extra")
                if n.ins.sync_info is None:
                    n.ins.sync_info = mybir.SyncInfo(on_wait=[wx], on_update=[])
                else:
                    n.ins.sync_info.on_wait = [wx]
        self.nc.sync.drain()
        self.nc.all_engine_barrier()
        assert self.sems is not None
        popped = self.nc._tile_sem_poison_stack.pop()
        assert popped is self._sem_poison
        self.nc.clear_and_free_semaphores(list(self.sems.allocated().values()))
        self.nc.all_engine_barrier()

    tile.TileContext._drain_and_barrier = _drain_and_barrier
    tile.TileContext._lstm_patched = True


def _split_syncs(nc, max_waits=1, max_updates=1):
    import concourse.mybir as mybir

    ctr = [0]

    def mknop(engine, waits, updates):
        ctr[0] += 1
        return mybir.InstNoOp(
            name=f"syncfix-{ctr[0]}",
            engine=engine,
            sync_info=mybir.SyncInfo(on_wait=list(waits), on_update=list(updates)),
        )

    for f in nc.m.functions:
        for bb in f.blocks:
            changed = False
            out = []
            for inst in bb.instructions:
                si = inst.sync_info
                if si is None or inst.engine == mybir.EngineType.Unassigned:
                    out.append(inst)
                    continue
                waits = list(si.on_wait or [])
                updates = list(si.on_update or [])
                if len(waits) <= max_waits and len(updates) <= max_updates:
                    out.append(inst)
                    continue
                changed = True
                for wx in waits[:-max_waits] if max_waits else waits:
                    out.append(mknop(inst.engine, [wx], []))
                si.on_wait = waits[-max_waits:] if max_waits else []
                extra_u = updates[max_updates:] if max_updates else updates
                si.on_update = updates[:max_updates] if max_updates else []
                out.append(inst)
                for ux in extra_u:
                    out.append(mknop(inst.engine, [], [ux]))
            if changed:
                bb.instructions = out
    return nc


def _build(KL=KL, MB=MB, PW=PW):
    import concourse.bass as bass
    import concourse.mybir as mybir
    import concourse.tile as tile
    from contextlib import ExitStack

    KM = KL * MB
    TC = -(-T // KM)
    TP = KM * TC
    NS = TC + PW
    TS = NS + 1
    KB = KL * BL

    _patch_tile_drain()
    F32 = mybir.dt.float32
    BF16 = mybir.dt.bfloat16
    SIG = mybir.ActivationFunctionType.Sigmoid
    ADD = mybir.AluOpType.add
    MUL = mybir.AluOpType.mult

    nc = bass.Bass()
    xt = nc.dram_tensor("xt", [2, 128, (TP + PW) * BL], BF16, kind="ExternalInput")
    wt = nc.dram_tensor("wt", [2, 128, G], BF16, kind="ExternalInput")
    rt = nc.dram_tensor("rt", [2, 128, G], BF16, kind="ExternalInput")
    bcw = nc.dram_tensor("bcw", [1, 2 * 128], F32, kind="ExternalInput")
    out = nc.dram_tensor("out", [2, 128, BL, T], BF16, kind="ExternalOutput")

    with ExitStack() as ctx:
        tc = ctx.enter_context(tile.TileContext(nc))
        const = ctx.enter_context(tc.tile_pool(name="const", bufs=1))
        gpool = ctx.enter_context(tc.tile_pool(name="g", bufs=1, space="PSUM"))

        wb = const.tile([128, 2, G], BF16)
        rb = const.tile([128, 2, G], BF16)
        bias_w = const.tile([128, 2, 128], BF16)
        bcs = const.tile([128, 2 * 128], F32)
        ones = const.tile([128, KB], BF16)

        for kx in range(2):
            nc.sync.dma_start(out=wb[:, kx, :], in_=wt[kx, :, :])
            nc.sync.dma_start(out=rb[:, kx, :], in_=rt[kx, :, :])
        nc.sync.dma_start(out=bcs[0:1, :], in_=bcw[:, :])
        nc.vector.memset(bias_w[:, :, :], 0.0)
        nc.scalar.copy(bias_w[0:1, :, :], bcs[0:1, :])
        nc.vector.memset(ones[:, :], 0.0)
        nc.vector.memset(ones[0:1, :], 1.0)

        x_sb = const.tile([128, 2, KM, NS * BL], BF16)
        for kx in range(2):
            for q in range(KM):
                nc.sync.dma_start(
                    out=x_sb[:, kx, q, :],
                    in_=xt[kx, :, q * TC * BL:(q * TC + NS) * BL],
                )

        hbuf, ut, at, bt, st, cp = [], [], [], [], [], []
        for b in range(MB):
            hbuf.append(const.tile([128, 2, KL, BL, TS], BF16, name=f"hbuf{b}"))
            ut.append(const.tile([128, 8, KB], BF16, name=f"u{b}"))
            at.append(const.tile([128, 2, KB], BF16, name=f"a{b}"))
            bt.append(const.tile([128, 2, KB], BF16, name=f"b{b}"))
            st.append(const.tile([128, 2, 2, KB], BF16, name=f"s{b}"))
            cp.append(const.tile([128, 2, 2, KB], BF16, name=f"cp{b}"))
            nc.vector.memset(hbuf[b][:, :, :, :, 0], 0.0)
            nc.vector.memset(cp[b][:, :, :, :], 0.0)

        gt = [[gpool.tile([128, 8, KB], F32, name=f"g{b}_{par}")
               for par in range(2)] for b in range(MB)]

        for tau in range(NS):
            for b in range(MB):
                if b == 0 and tau == PW:
                    # chunk 0 must start t=0 from true zero state (its
                    # "warmup" ran on the zero-padded x region)
                    nc.vector.memset(cp[0][:, 0, :, 0:BL], 0.0)
                    nc.vector.memset(hbuf[0][:, :, 0, :, PW], 0.0)
                g = gt[b][tau % 2]
                # HW quirk: a PSUM region's accumulation chain must be
                # consecutive on the PE — interleaving open groups across
                # regions clobbers partial sums.
                for c in range(8):
                    for kx in range(2):
                        nc.tensor.matmul(
                            g[:, c, :],
                            wb[:, kx, c * 128:(c + 1) * 128],
                            x_sb[:, kx, b * KL:(b + 1) * KL,
                                 tau * BL:(tau + 1) * BL],
                            start=(kx == 0), stop=False, skip_group_check=True,
                        )
                    if c >= 6:
                        nc.tensor.matmul(
                            g[:, c, :], bias_w[:, c - 6, :], ones[:, :],
                            start=False, stop=False, skip_group_check=True,
                        )
                    for kh in range(2):
                        nc.tensor.matmul(
                            g[:, c, :],
                            rb[:, kh, c * 128:(c + 1) * 128],
                            hbuf[b][:, kh, :, :, tau],
                            start=False, stop=(kh == 1), skip_group_check=True,
                        )
                # u = sigmoid over all gates [i(0:2) f(2:4) o(4:6) cand(6:8)]
                nc.scalar.activation(ut[b][:, :, :], g[:, :, :], SIG)
                # f*c
                nc.vector.tensor_mul(
                    bt[b][:, :, :], ut[b][:, 2:4, :], cp[b][:, 0, :, :])
                # i*cand/2 = (u_c - .5)*u_i
                nc.vector.scalar_tensor_tensor(
                    at[b][:, :, :], ut[b][:, 6:8, :], -0.5, ut[b][:, 0:2, :],
                    ADD, MUL)
                # s = 2*(i*cand/2) + f*c
                nc.vector.scalar_tensor_tensor(
                    st[b][:, 0, :, :], at[b][:, :, :], 2.0, bt[b][:, :, :],
                    MUL, ADD)
                # s2 = AL*s + BE
                nc.vector.tensor_scalar(
                    st[b][:, 1, :, :], st[b][:, 0, :, :], AL_PHI, BE_PHI,
                    MUL, ADD)
                # [c' | phi] = sigmoid([s | s2])
                nc.scalar.activation(
                    cp[b][:, :, :, :], st[b][:, :, :, :], SIG)
                # h = (phi*K_PHI)*o  -> bf16 h history (also the output)
                nc.vector.scalar_tensor_tensor(
                    hbuf[b][:, :, :, :, tau + 1].rearrange(
                        "p u k b -> p u (k b)"),
                    cp[b][:, 1, :, :], K_PHI, ut[b][:, 4:6, :],
                    MUL, MUL)

        for b in range(MB):
            for j in range(KL):
                q = b * KL + j
                t0 = q * TC
                tcv = min(TC, T - t0)
                if tcv <= 0:
                    continue
                for kp in range(2):
                    nc.sync.dma_start(
                        out=out[kp, :, :, t0:t0 + tcv],
                        in_=hbuf[b][:, kp, j, :, PW + 1:PW + 1 + tcv],
                    )
    _split_syncs(nc)
    return nc


def _prep_weights(Wd, Rd, bcd):
    import ml_dtypes
    Wp = np.ascontiguousarray(Wd).astype(np.float32).copy()
    Rp = np.ascontiguousarray(Rd).astype(np.float32).copy()
    Wp[:, 3 * U:] *= 2.0  # candidate pre-act doubled: tanh(a)=2*sig(2a)-1
    Rp[:, 3 * U:] *= 2.0
    wt = Wp.reshape(2, 128, G).astype(ml_dtypes.bfloat16)
    rt = Rp.reshape(2, 128, G).astype(ml_dtypes.bfloat16)
    bcw = (2.0 * np.asarray(bcd, np.float32)).reshape(1, 256)
    return wt, rt, np.ascontiguousarray(bcw)


def kernel(x, W_f, R_f, bc_f, W_b, R_b, bc_b):
    import ml_dtypes
    from concourse.bass_utils import run_bass_kernel_spmd

    x = np.asarray(x, dtype=np.float32)
    if "nc" not in _CACHE:
        _CACHE["nc"] = _build()
    nc = _CACHE["nc"]

    wtf, rtf, bcwf = _prep_weights(W_f, R_f, bc_f)
    wtb, rtb, bcwb = _prep_weights(W_b, R_b, bc_b)

    in_maps = []
    for core in range(8):
        fwd = core < 4
        b0 = (core % 4) * BL
        xs = x[b0:b0 + BL]
        if not fwd:
            xs = xs[:, ::-1, :]
        xp = np.zeros((BL, PW + TP, D), np.float32)
        xp[:, PW:PW + T] = xs
        xtr = np.ascontiguousarray(xp.transpose(2, 1, 0)).reshape(
            2, 128, (PW + TP) * BL).astype(ml_dtypes.bfloat16)
        in_maps.append({
            "xt": xtr,
            "wt": wtf if fwd else wtb,
            "rt": rtf if fwd else rtb,
            "bcw": bcwf if fwd else bcwb,
        })

    res = run_bass_kernel_spmd(nc, in_maps, core_ids=list(range(8)))

    outp = np.empty((16, T, 2 * U), dtype=np.float32)
    for core in range(8):
        od = np.asarray(res.results[core]["out"]).astype(np.float32)
        # od [2(kp), 128, BL, T] -> [BL, T, 256]
        hb = od.transpose(2, 3, 0, 1).reshape(BL, T, U)
        b0 = (core % 4) * BL
        if core < 4:
            outp[b0:b0 + BL, :, 0:U] = hb
        else:
            outp[b0:b0 + BL, :, U:2 * U] = hb
    return outp


# revision 21
# speedup vs baseline: 17.5392x; 1.6698x over previous
"""BiLSTM (B=16, T=2048, D=U=256) on 8 TRN2 NeuronCores.

Sharding: 8 cores = 2 directions x 4 batch-shards (B_local=4 per core).
Backward cores receive x time-reversed on the host; all cores run the same
SPMD program (a forward scan), so no collectives are needed.

Chunked-parallel scan: the cell update c' = sigmoid(f*c + i*cand) is
strongly contracting (|dc'/dc| <= f/4), so state influence decays ~7x per
step.  T is therefore split into KM chunks, each warmed up from zero state
over the P steps preceding its range (error ~1e-6 at P=8).  The chunks run
as M bundles of K lanes: the K lanes of a bundle advance in lockstep, so
every engine instruction covers all K lanes at once (amortizing per-
instruction fixed costs), while the M bundles interleave in program order
to hide the ~2us per-step cross-engine latency chain.

Per fused step: PE accumulates x_t@W (lanes strided across chunks, zero-
padded left edge covers warmup before t=0), a rank-1 bias injection for
the candidate gate, and h@R into one PSUM tile; ScalarE applies one
sigmoid over all 4*K gate groups (candidate columns pre-doubled so
tanh(a) = 2*sigmoid(2a)-1); DVE computes the cell update in 4 fused ops;
a second ScalarE sigmoid yields [c' | phi] jointly where
tanh(sigmoid(s)) ~= K_PHI*sigmoid(AL*s+BE); one last DVE op writes
h = K_PHI*phi*o straight into the bf16 h-history that doubles as the
DMA-out staging buffer (output dram is bf16; host converts to f32).
"""

import numpy as np

_CACHE = {}

T = 2048
D = 256
U = 256
G = 4 * U
BL = 4  # batch per core

KL = 12  # lanes (chunks) per bundle, fused per instruction
MB = 4   # bundles, staggered to hide latency
PW = 2   # warmup steps per chunk
KM = KL * MB
TC = -(-T // KM)          # chunk length (T padded up to KM*TC)
TP = KM * TC
NS = TC + PW              # steps per chain
TS = NS + 1               # h slots (slot 0 = initial zeros)
KB = KL * BL              # fused free width per gate chunk

K_PHI = 0.7589144336406901
AL_PHI = 1.0834263081088795
BE_PHI = 0.44379053813456204


def _patch_tile_drain():
    """This container's walrus accepts only one sem-wait/update per
    instruction; spread Tile's final-drain waits across NOPs."""
    import concourse.tile as tile
    import concourse.mybir as mybir
    from concourse.vector_clock import ScopedClock

    if getattr(tile.TileContext, "_lstm_patched", False):
        return

    def _drain_and_barrier(self, tick_clock, wait_clock):
        carrier = self.nc.sync.nop(nofuse=True, hint="final_wait_carrier")
        wait_clock.add_sem_waits(
            carrier.ins, ScopedClock({None: tick_clock.global_clock})
        )
        si = carrier.ins.sync_info
        waits = list(si.on_wait or []) if si is not None else []
        if len(waits) > 1:
            si.on_wait = waits[:1]
            for wx in waits[1:]:
                n = self.nc.sync.nop(nofuse=True, hint="final_wait_extra")
                if n.ins.sync_info is None:
                    n.ins.sync_info = mybir.SyncInfo(on_wait=[wx], on_update=[])
                else:
                    n.ins.sync_info.on_wait = [wx]
        self.nc.sync.drain()
        self.nc.all_engine_barrier()
        assert self.sems is not None
        popped = self.nc._tile_sem_poison_stack.pop()
        assert popped is self._sem_poison
        self.nc.clear_and_free_semaphores(list(self.sems.allocated().values()))
        self.nc.all_engine_barrier()

    tile.TileContext._drain_and_barrier = _drain_and_barrier
    tile.TileContext._lstm_patched = True


def _split_syncs(nc, max_waits=1, max_updates=1):
    import concourse.mybir as mybir

    ctr = [0]

    def mknop(engine, waits, updates):
        ctr[0] += 1
        return mybir.InstNoOp(
            name=f"syncfix-{ctr[0]}",
            engine=engine,
            sync_info=mybir.SyncInfo(on_wait=list(waits), on_update=list(updates)),
        )

    for f in nc.m.functions:
        for bb in f.blocks:
            changed = False
            out = []
            for inst in bb.instructions:
                si = inst.sync_info
                if si is None or inst.engine == mybir.EngineType.Unassigned:
                    out.append(inst)
                    continue
                waits = list(si.on_wait or [])
                updates = list(si.on_update or [])
                if len(waits) <= max_waits and len(updates) <= max_updates:
                    out.append(inst)
                    continue
                changed = True
                for wx in waits[:-max_waits] if max_waits else waits:
                    out.append(mknop(inst.engine, [wx], []))
                si.on_wait = waits[-max_waits:] if max_waits else []
                extra_u = updates[max_updates:] if max_updates else updates
                si.on_update = updates[:max_updates] if max_updates else []
                out.append(inst)
                for ux in extra_u:
                    out.append(mknop(inst.engine, [], [ux]))
            if changed:
                bb.instructions = out
    return nc


def _build(KL=KL, MB=MB, PW=PW, PAR=2, SPLIT=False):
    import concourse.bass as bass
    import concourse.mybir as mybir
    import concourse.tile as tile
    from contextlib import ExitStack

    KM = KL * MB
    TC = -(-T // KM)
    TP = KM * TC
    NS = TC + PW
    TS = NS + 1
    KB = KL * BL

    _patch_tile_drain()
    F32 = mybir.dt.float32
    BF16 = mybir.dt.bfloat16
    SIG = mybir.ActivationFunctionType.Sigmoid
    ADD = mybir.AluOpType.add
    MUL = mybir.AluOpType.mult

    nc = bass.Bass()
    xt = nc.dram_tensor("xt", [2, 128, (TP + PW) * BL], BF16, kind="ExternalInput")
    wt = nc.dram_tensor("wt", [2, 128, G], BF16, kind="ExternalInput")
    rt = nc.dram_tensor("rt", [2, 128, G], BF16, kind="ExternalInput")
    bcw = nc.dram_tensor("bcw", [1, 2 * 128], F32, kind="ExternalInput")
    out = nc.dram_tensor("out", [2, 128, MB, KL * BL * TS], BF16,
                         kind="ExternalOutput")

    with ExitStack() as ctx:
        tc = ctx.enter_context(tile.TileContext(nc))
        const = ctx.enter_context(tc.tile_pool(name="const", bufs=1))
        gpool = ctx.enter_context(tc.tile_pool(name="g", bufs=1, space="PSUM"))

        wb = const.tile([128, 2, G], BF16)
        rb = const.tile([128, 2, G], BF16)
        bias_w = const.tile([128, 2, 128], BF16)
        bcs = const.tile([128, 2 * 128], F32)
        ones = const.tile([128, KB], BF16)

        for kx in range(2):
            nc.sync.dma_start(out=wb[:, kx, :], in_=wt[kx, :, :])
            nc.sync.dma_start(out=rb[:, kx, :], in_=rt[kx, :, :])
        nc.sync.dma_start(out=bcs[0:1, :], in_=bcw[:, :])
        nc.vector.memset(bias_w[:, :, :], 0.0)
        nc.scalar.copy(bias_w[0:1, :, :], bcs[0:1, :])
        nc.vector.memset(ones[:, :], 0.0)
        nc.vector.memset(ones[0:1, :], 1.0)

        x_sb = const.tile([128, 2, KM, NS * BL], BF16)
        # stream x in tau-waves: one strided DMA per (kx, wave) covers that
        # tau-segment of every chunk, so compute starts after the first
        # small wave instead of after the full x load
        SW = 8
        for kx in range(2):
            for w0 in range(0, TC, SW):
                w1 = min(w0 + SW, TC)
                nc.sync.dma_start(
                    out=x_sb[:, kx, :, w0 * BL:w1 * BL],
                    in_=xt[kx, :, :KM * TC * BL].rearrange(
                        "p (q c) -> p q c", q=KM)[:, :, w0 * BL:w1 * BL],
                )
        # warmup-overlap tails [TC, NS) per chunk (needed only at the end
        # of each chain's range)
        for kx in range(2):
            for q in range(KM):
                nc.sync.dma_start(
                    out=x_sb[:, kx, q, TC * BL:NS * BL],
                    in_=xt[kx, :, (q + 1) * TC * BL:(q * TC + NS) * BL],
                )

        hbuf, ut, at, bt, st, cp = [], [], [], [], [], []
        for b in range(MB):
            hbuf.append(const.tile([128, 2, KL, BL, TS], BF16, name=f"hbuf{b}"))
            ut.append([const.tile([128, 8, KB], BF16, name=f"u{b}_{p}")
                       for p in range(2)])
            at.append([const.tile([128, 2, KB], BF16, name=f"a{b}_{p}")
                       for p in range(2)])
            bt.append([const.tile([128, 2, KB], BF16, name=f"b{b}_{p}")
                       for p in range(2)])
            st.append([const.tile([128, 2, 2, KB], BF16, name=f"s{b}_{p}")
                       for p in range(2)])
            cp.append([const.tile([128, 2, 2, KB], BF16, name=f"cp{b}_{p}")
                       for p in range(2)])
            nc.vector.memset(hbuf[b][:, :, :, :, 0], 0.0)
            for p in range(2):
                nc.vector.memset(cp[b][p][:, :, :, :], 0.0)

        gt = [[gpool.tile([128, 8, KB], F32, name=f"g{b}_{par}")
               for par in range(PAR)] for b in range(MB)]

        # PSUM region slot -> gate chunk; cand,i,f first so the gate
        # sigmoid can fire before the o-chunk matmuls finish
        SL2CH = [6, 7, 0, 1, 2, 3, 4, 5]
        # ut slots: cand 0:2 | i 2:4 | f 4:6 | o 6:8

        def region(nc, g, b, tau, s):
            ch = SL2CH[s]
            for kx in range(2):
                nc.tensor.matmul(
                    g[:, s, :],
                    wb[:, kx, ch * 128:(ch + 1) * 128],
                    x_sb[:, kx, b * KL:(b + 1) * KL,
                         tau * BL:(tau + 1) * BL],
                    start=(kx == 0), stop=False, skip_group_check=True,
                )
            if ch >= 6:
                nc.tensor.matmul(
                    g[:, s, :], bias_w[:, ch - 6, :], ones[:, :],
                    start=False, stop=False, skip_group_check=True,
                )
            for kh in range(2):
                nc.tensor.matmul(
                    g[:, s, :],
                    rb[:, kh, ch * 128:(ch + 1) * 128],
                    hbuf[b][:, kh, :, :, tau],
                    start=False, stop=(kh == 1), skip_group_check=True,
                )

        for tau in range(NS):
            for b in range(MB):
                pc, pp = tau % 2, (tau + 1) % 2
                u, cpc, cpp = ut[b][pc], cp[b][pc], cp[b][pp]
                if b == 0 and tau == PW:
                    # chunk 0 must start t=0 from true zero state (its
                    # "warmup" ran on the zero-padded x region)
                    nc.vector.memset(cpp[:, 0, :, 0:BL], 0.0)
                    nc.vector.memset(hbuf[0][:, :, 0, :, PW], 0.0)
                g = gt[b][tau % PAR]
                # HW quirk: a PSUM region's accumulation chain must be
                # consecutive on the PE — interleaving open groups across
                # regions clobbers partial sums.
                if SPLIT:
                    for s in range(6):
                        region(nc, g, b, tau, s)
                    # sigmoid over [cand|i|f] as soon as their regions close
                    nc.scalar.activation(u[:, 0:6, :], g[:, 0:6, :], SIG)
                    for s in (6, 7):
                        region(nc, g, b, tau, s)
                    nc.scalar.activation(u[:, 6:8, :], g[:, 6:8, :], SIG)
                else:
                    for s in range(8):
                        region(nc, g, b, tau, s)
                    nc.scalar.activation(u[:, :, :], g[:, :, :], SIG)
                # i*cand/2 = (u_c - .5)*u_i
                nc.vector.scalar_tensor_tensor(
                    at[b][pc][:, :, :], u[:, 0:2, :], -0.5, u[:, 2:4, :],
                    ADD, MUL)
                # f*c
                nc.vector.tensor_mul(
                    bt[b][pc][:, :, :], u[:, 4:6, :], cpp[:, 0, :, :])
                # s = 2*(i*cand/2) + f*c
                nc.vector.scalar_tensor_tensor(
                    st[b][pc][:, 0, :, :], at[b][pc][:, :, :], 2.0,
                    bt[b][pc][:, :, :], MUL, ADD)
                # s2 = AL*s + BE
                nc.vector.tensor_scalar(
                    st[b][pc][:, 1, :, :], st[b][pc][:, 0, :, :],
                    AL_PHI, BE_PHI, MUL, ADD)
                # [c' | phi] = sigmoid([s | s2])
                nc.scalar.activation(
                    cpc[:, :, :, :], st[b][pc][:, :, :, :], SIG)
                # h/K_PHI = phi*o -> bf16 h history (also the output).
                # Runs on the otherwise-idle GPSIMD engine (which only
                # supports plain tensor_tensor); the K_PHI scale is folded
                # into R on the host and into the output decode.
                nc.gpsimd.tensor_mul(
                    hbuf[b][:, :, :, :, tau + 1].rearrange(
                        "p u k b -> p u (k b)"),
                    cpc[:, 1, :, :], u[:, 6:8, :])
        # one fully-contiguous dump per (bundle, kp): 128 descriptors each
        for b in range(MB):
            for kp in range(2):
                nc.sync.dma_start(
                    out=out[kp, :, b, :],
                    in_=hbuf[b][:, kp, :, :, :].rearrange(
                        "p k b t -> p (k b t)"),
                )
    _split_syncs(nc)
    return nc


def _prep_weights(Wd, Rd, bcd):
    import ml_dtypes
    Wp = np.ascontiguousarray(Wd).astype(np.float32).copy()
    Rp = np.ascontiguousarray(Rd).astype(np.float32).copy()
    Wp[:, 3 * U:] *= 2.0  # candidate pre-act doubled: tanh(a)=2*sig(2a)-1
    Rp[:, 3 * U:] *= 2.0
    Rp *= K_PHI           # h is stored as h/K_PHI; R absorbs the scale
    wt = Wp.reshape(2, 128, G).astype(ml_dtypes.bfloat16)
    rt = Rp.reshape(2, 128, G).astype(ml_dtypes.bfloat16)
    bcw = (2.0 * np.asarray(bcd, np.float32)).reshape(1, 256)
    return wt, rt, np.ascontiguousarray(bcw)


def kernel(x, W_f, R_f, bc_f, W_b, R_b, bc_b):
    import ml_dtypes
    from concourse.bass_utils import run_bass_kernel_spmd

    x = np.asarray(x, dtype=np.float32)
    if "nc" not in _CACHE:
        _CACHE["nc"] = _build()
    nc = _CACHE["nc"]

    wtf, rtf, bcwf = _prep_weights(W_f, R_f, bc_f)
    wtb, rtb, bcwb = _prep_weights(W_b, R_b, bc_b)

    in_maps = []
    for core in range(8):
        fwd = core < 4
        b0 = (core % 4) * BL
        xs = x[b0:b0 + BL]
        if not fwd:
            xs = xs[:, ::-1, :]
        xp = np.zeros((BL, PW + TP, D), np.float32)
        xp[:, PW:PW + T] = xs
        xtr = np.ascontiguousarray(xp.transpose(2, 1, 0)).reshape(
            2, 128, (PW + TP) * BL).astype(ml_dtypes.bfloat16)
        in_maps.append({
            "xt": xtr,
            "wt": wtf if fwd else wtb,
            "rt": rtf if fwd else rtb,
            "bcw": bcwf if fwd else bcwb,
        })

    res = run_bass_kernel_spmd(nc, in_maps, core_ids=list(range(8)))

    TC0 = -(-T // (KL * MB))
    outp = np.empty((16, T, 2 * U), dtype=np.float32)
    for core in range(8):
        od = np.asarray(res.results[core]["out"]).astype(np.float32)
        TS0 = TC0 + PW + 1
        od = od.reshape(2, 128, MB, KL, BL, TS0)[..., PW + 1:PW + 1 + TC0]
        # [kp, p, b, j, b4, t'] -> [b4, (b j t'), kp*128+p]
        hb = od.transpose(4, 2, 3, 5, 0, 1).reshape(BL, -1, U)[:, :T] * K_PHI
        b0 = (core % 4) * BL
        if core < 4:
            outp[b0:b0 + BL, :, 0:U] = hb
        else:
            outp[b0:b0 + BL, :, U:2 * U] = hb
    return outp


# revision 24
# speedup vs baseline: 19.2371x; 1.0968x over previous
"""BiLSTM (B=16, T=2048, D=U=256) on 8 TRN2 NeuronCores.

Sharding: 8 cores = 2 directions x 4 batch-shards (B_local=4 per core).
Backward cores receive x time-reversed on the host; all cores run the same
SPMD program (a forward scan), so no collectives are needed.

Chunked-parallel scan: the cell update c' = sigmoid(f*c + i*cand) is
strongly contracting (|dc'/dc| <= f/4), so state influence decays ~7x per
step.  T is therefore split into KM chunks, each warmed up from zero state
over the P steps preceding its range (error ~1e-6 at P=8).  The chunks run
as M bundles of K lanes: the K lanes of a bundle advance in lockstep, so
every engine instruction covers all K lanes at once (amortizing per-
instruction fixed costs), while the M bundles interleave in program order
to hide the ~2us per-step cross-engine latency chain.

Per fused step: PE accumulates x_t@W (lanes strided across chunks, zero-
padded left edge covers warmup before t=0), a rank-1 bias injection for
the candidate gate, and h@R into one PSUM tile; ScalarE applies one
sigmoid over all 4*K gate groups (candidate columns pre-doubled so
tanh(a) = 2*sigmoid(2a)-1); DVE computes the cell update in 4 fused ops;
a second ScalarE sigmoid yields [c' | phi] jointly where
tanh(sigmoid(s)) ~= K_PHI*sigmoid(AL*s+BE); one last DVE op writes
h = K_PHI*phi*o straight into the bf16 h-history that doubles as the
DMA-out staging buffer (output dram is bf16; host converts to f32).
"""

import numpy as np

_CACHE = {}

T = 2048
D = 256
U = 256
G = 4 * U
BL = 4  # batch per core

KL = 24  # lanes (chunks) per bundle, fused per instruction
MB = 4   # bundles, staggered to hide latency
PW = 2   # warmup steps per chunk
KM = KL * MB
TC = -(-T // KM)          # chunk length (T padded up to KM*TC)
TP = KM * TC
NS = TC + PW              # steps per chain
TS = NS + 1               # h slots (slot 0 = initial zeros)
KB = KL * BL              # fused free width per gate chunk

# minimax deg-3 fit of tanh(sigmoid(s)) on s in [-1.02, 1.88]
# (s = f*c + i*cand is mathematically confined to (-1, 2))
PC0 = 0.4619294218978857
PC1 = 0.19082902146374442
PC2 = -0.020634543916420812
PC3 = -0.0078868162842547


def _patch_tile_drain():
    """This container's walrus accepts only one sem-wait/update per
    instruction; spread Tile's final-drain waits across NOPs."""
    import concourse.tile as tile
    import concourse.mybir as mybir
    from concourse.vector_clock import ScopedClock

    if getattr(tile.TileContext, "_lstm_patched", False):
        return

    def _drain_and_barrier(self, tick_clock, wait_clock):
        carrier = self.nc.sync.nop(nofuse=True, hint="final_wait_carrier")
        wait_clock.add_sem_waits(
            carrier.ins, ScopedClock({None: tick_clock.global_clock})
        )
        si = carrier.ins.sync_info
        waits = list(si.on_wait or []) if si is not None else []
        if len(waits) > 1:
            si.on_wait = waits[:1]
            for wx in waits[1:]:
                n = self.nc.sync.nop(nofuse=True, hint="final_wait_extra")
                if n.ins.sync_info is None:
                    n.ins.sync_info = mybir.SyncInfo(on_wait=[wx], on_update=[])
                else:
                    n.ins.sync_info.on_wait = [wx]
        self.nc.sync.drain()
        self.nc.all_engine_barrier()
        assert self.sems is not None
        popped = self.nc._tile_sem_poison_stack.pop()
        assert popped is self._sem_poison
        self.nc.clear_and_free_semaphores(list(self.sems.allocated().values()))
        self.nc.all_engine_barrier()

    tile.TileContext._drain_and_barrier = _drain_and_barrier
    tile.TileContext._lstm_patched = True


def _split_syncs(nc, max_waits=1, max_updates=1):
    import concourse.mybir as mybir

    ctr = [0]

    def mknop(engine, waits, updates):
        ctr[0] += 1
        return mybir.InstNoOp(
            name=f"syncfix-{ctr[0]}",
            engine=engine,
            sync_info=mybir.SyncInfo(on_wait=list(waits), on_update=list(updates)),
        )

    for f in nc.m.functions:
        for bb in f.blocks:
            changed = False
            out = []
            for inst in bb.instructions:
                si = inst.sync_info
                if si is None or inst.engine == mybir.EngineType.Unassigned:
                    out.append(inst)
                    continue
                waits = list(si.on_wait or [])
                updates = list(si.on_update or [])
                if len(waits) <= max_waits and len(updates) <= max_updates:
                    out.append(inst)
                    continue
                changed = True
                for wx in waits[:-max_waits] if max_waits else waits:
                    out.append(mknop(inst.engine, [wx], []))
                si.on_wait = waits[-max_waits:] if max_waits else []
                extra_u = updates[max_updates:] if max_updates else updates
                si.on_update = updates[:max_updates] if max_updates else []
                out.append(inst)
                for ux in extra_u:
                    out.append(mknop(inst.engine, [], [ux]))
            if changed:
                bb.instructions = out
    return nc


def _build(KL=KL, MB=MB, PW=PW, PAR=1, SPLIT=False):
    import concourse.bass as bass
    import concourse.mybir as mybir
    import concourse.tile as tile
    from contextlib import ExitStack

    KM = KL * MB
    TC = -(-T // KM)
    TP = KM * TC
    NS = TC + PW
    TS = NS + 1
    KB = KL * BL

    _patch_tile_drain()
    F32 = mybir.dt.float32
    BF16 = mybir.dt.bfloat16
    SIG = mybir.ActivationFunctionType.Sigmoid
    ADD = mybir.AluOpType.add
    MUL = mybir.AluOpType.mult

    nc = bass.Bass()
    xt = nc.dram_tensor("xt", [2, 128, (TP + PW) * BL], BF16, kind="ExternalInput")
    wt = nc.dram_tensor("wt", [2, 128, G], BF16, kind="ExternalInput")
    rt = nc.dram_tensor("rt", [2, 128, G], BF16, kind="ExternalInput")
    bcw = nc.dram_tensor("bcw", [1, 2 * 128], F32, kind="ExternalInput")
    out = nc.dram_tensor("out", [2, 128, MB, KL * BL * TS], BF16,
                         kind="ExternalOutput")

    with ExitStack() as ctx:
        tc = ctx.enter_context(tile.TileContext(nc))
        const = ctx.enter_context(tc.tile_pool(name="const", bufs=1))
        gpool = ctx.enter_context(tc.tile_pool(name="g", bufs=1, space="PSUM"))

        wb = const.tile([128, 2, G], BF16)
        rb = const.tile([128, 2, G], BF16)
        bias_w = const.tile([128, 2, 128], BF16)
        bcs = const.tile([128, 2 * 128], F32)
        ones = const.tile([128, KB], BF16)

        for kx in range(2):
            nc.sync.dma_start(out=wb[:, kx, :], in_=wt[kx, :, :])
            nc.sync.dma_start(out=rb[:, kx, :], in_=rt[kx, :, :])
        nc.sync.dma_start(out=bcs[0:1, :], in_=bcw[:, :])
        nc.vector.memset(bias_w[:, :, :], 0.0)
        nc.scalar.copy(bias_w[0:1, :, :], bcs[0:1, :])
        nc.vector.memset(ones[:, :], 0.0)
        nc.vector.memset(ones[0:1, :], 1.0)

        x_sb = const.tile([128, 2, KM, NS * BL], BF16)
        # stream x in tau-waves: one strided DMA per (kx, wave) covers that
        # tau-segment of every chunk, so compute starts after the first
        # small wave instead of after the full x load
        SW = 8
        for kx in range(2):
            for w0 in range(0, TC, SW):
                w1 = min(w0 + SW, TC)
                nc.sync.dma_start(
                    out=x_sb[:, kx, :, w0 * BL:w1 * BL],
                    in_=xt[kx, :, :KM * TC * BL].rearrange(
                        "p (q c) -> p q c", q=KM)[:, :, w0 * BL:w1 * BL],
                )
        # warmup-overlap tails [TC, NS) per chunk (needed only at the end
        # of each chain's range)
        for kx in range(2):
            for q in range(KM):
                nc.sync.dma_start(
                    out=x_sb[:, kx, q, TC * BL:NS * BL],
                    in_=xt[kx, :, (q + 1) * TC * BL:(q * TC + NS) * BL],
                )

        hbuf, ut, at, bt, st, cp = [], [], [], [], [], []
        for b in range(MB):
            hbuf.append(const.tile([128, 2, KL, BL, TS], BF16, name=f"hbuf{b}"))
            ut.append([const.tile([128, 10, KB], BF16, name=f"u{b}_{p}")
                       for p in range(2)])
            at.append([const.tile([128, 2, KB], BF16, name=f"a{b}_{p}")
                       for p in range(2)])
            bt.append([const.tile([128, 2, KB], BF16, name=f"b{b}_{p}")
                       for p in range(2)])
            st.append([[const.tile([128, 2, KB], BF16, name=f"pt{b}_{p}_{i}")
                        for i in range(5)] for p in range(2)])
            cp.append([const.tile([128, 2, KB], BF16, name=f"ph{b}_{p}")
                       for p in range(2)])
            nc.vector.memset(hbuf[b][:, :, :, :, 0], 0.0)

        gt = [[gpool.tile([128, 10, KB], F32, name=f"g{b}_{par}")
               for par in range(PAR)] for b in range(MB)]

        # PSUM region slot -> gate chunk; cand,i,f first so the gate
        # sigmoid can fire before the o-chunk matmuls finish
        SL2CH = [6, 7, 0, 1, 2, 3, 4, 5]
        # ut slots: cand 0:2 | i 2:4 | f 4:6 | o 6:8

        def region(nc, g, b, tau, s):
            ch = SL2CH[s]
            for kx in range(2):
                nc.tensor.matmul(
                    g[:, s, :],
                    wb[:, kx, ch * 128:(ch + 1) * 128],
                    x_sb[:, kx, b * KL:(b + 1) * KL,
                         tau * BL:(tau + 1) * BL],
                    start=(kx == 0), stop=False, skip_group_check=True,
                )
            if ch >= 6:
                nc.tensor.matmul(
                    g[:, s, :], bias_w[:, ch - 6, :], ones[:, :],
                    start=False, stop=False, skip_group_check=True,
                )
            for kh in range(2):
                nc.tensor.matmul(
                    g[:, s, :],
                    rb[:, kh, ch * 128:(ch + 1) * 128],
                    hbuf[b][:, kh, :, :, tau],
                    start=False, stop=(kh == 1), skip_group_check=True,
                )

        for tau in range(NS):
            for b in range(MB):
                pc = tau % 2
                u = ut[b][pc]
                g = gt[b][tau % PAR]
                gnext = gt[b][(tau + 1) % PAR]
                if b == 0 and tau == PW:
                    # chunk 0 must start t=0 from true zero state (its
                    # "warmup" ran on the zero-padded x region):
                    # force c'(PW) = sigmoid(-30) ~ 0 and h(PW-1) = 0
                    nc.vector.memset(g[:, 8:10, 0:BL], -30.0)
                    nc.vector.memset(hbuf[0][:, :, 0, :, PW], 0.0)
                # HW quirk: a PSUM region's accumulation chain must be
                # consecutive on the PE — interleaving open groups across
                # regions clobbers partial sums.
                for s in range(8):
                    region(nc, g, b, tau, s)
                # u = sigmoid over [cand|i|f|o | s(tau-1)] -> last two
                # slots give c' = sigmoid(s_prev), the carried cell state
                if tau == 0:
                    nc.scalar.activation(u[:, 0:8, :], g[:, 0:8, :], SIG)
                else:
                    nc.scalar.activation(u[:, :, :], g[:, :, :], SIG)
                # A = i*cand/2 = (u_c - .5)*u_i
                nc.vector.scalar_tensor_tensor(
                    at[b][pc][:, :, :], u[:, 0:2, :], -0.5, u[:, 2:4, :],
                    ADD, MUL)
                t_, b3_, a3_, c_, s_ = st[b][pc]
                sl = s_[:, :, :]
                if tau > 0:
                    # B = f*c ; s = 2A + B (bf16, SBUF)
                    nc.vector.tensor_mul(
                        bt[b][pc][:, :, :], u[:, 4:6, :], u[:, 8:10, :])
                    nc.vector.scalar_tensor_tensor(
                        sl, at[b][pc][:, :, :], 2.0,
                        bt[b][pc][:, :, :], MUL, ADD)
                else:
                    nc.vector.tensor_scalar(
                        sl, at[b][pc][:, :, :], 2.0, None, MUL)
                # stage s into the next bank's spare PSUM regions on ScalarE
                # (native PSUM port; DVE PSUM access is 2x pricier)
                nc.scalar.copy(gnext[:, 8:10, :], sl)
                # phi = tanh(sigmoid(s)) via deg-3 poly, all DVE-chained:
                # t=s^2; B3=c3*s+c2; A3=c1*s+c0; phi = A3 + t*B3
                ph = cp[b][pc]
                nc.vector.tensor_mul(t_[:, :, :], sl, sl)
                nc.vector.tensor_scalar(b3_[:, :, :], sl, PC3, PC2, MUL, ADD)
                nc.vector.tensor_scalar(a3_[:, :, :], sl, PC1, PC0, MUL, ADD)
                nc.vector.tensor_mul(c_[:, :, :], t_[:, :, :], b3_[:, :, :])
                nc.vector.tensor_add(ph[:, :, :], a3_[:, :, :], c_[:, :, :])
                # h = phi*o -> bf16 h history (also the output); on GPSIMD
                nc.gpsimd.tensor_mul(
                    hbuf[b][:, :, :, :, tau + 1].rearrange(
                        "p u k b -> p u (k b)"),
                    ph[:, :, :], u[:, 6:8, :])
        # one fully-contiguous dump per (bundle, kp): 128 descriptors each
        for b in range(MB):
            for kp in range(2):
                nc.sync.dma_start(
                    out=out[kp, :, b, :],
                    in_=hbuf[b][:, kp, :, :, :].rearrange(
                        "p k b t -> p (k b t)"),
                )
    _split_syncs(nc)
    return nc


def _prep_weights(Wd, Rd, bcd):
    import ml_dtypes
    Wp = np.ascontiguousarray(Wd).astype(np.float32).copy()
    Rp = np.ascontiguousarray(Rd).astype(np.float32).copy()
    Wp[:, 3 * U:] *= 2.0  # candidate pre-act doubled: tanh(a)=2*sig(2a)-1
    Rp[:, 3 * U:] *= 2.0
    wt = Wp.reshape(2, 128, G).astype(ml_dtypes.bfloat16)
    rt = Rp.reshape(2, 128, G).astype(ml_dtypes.bfloat16)
    bcw = (2.0 * np.asarray(bcd, np.float32)).reshape(1, 256)
    return wt, rt, np.ascontiguousarray(bcw)


def kernel(x, W_f, R_f, bc_f, W_b, R_b, bc_b):
    import ml_dtypes
    from concourse.bass_utils import run_bass_kernel_spmd

    x = np.asarray(x, dtype=np.float32)
    if "nc" not in _CACHE:
        _CACHE["nc"] = _build()
    nc = _CACHE["nc"]

    wtf, rtf, bcwf = _prep_weights(W_f, R_f, bc_f)
    wtb, rtb, bcwb = _prep_weights(W_b, R_b, bc_b)

    in_maps = []
    for core in range(8):
        fwd = core < 4
        b0 = (core % 4) * BL
        xs = x[b0:b0 + BL]
        if not fwd:
            xs = xs[:, ::-1, :]
        xp = np.zeros((BL, PW + TP, D), np.float32)
        xp[:, PW:PW + T] = xs
        xtr = np.ascontiguousarray(xp.transpose(2, 1, 0)).reshape(
            2, 128, (PW + TP) * BL).astype(ml_dtypes.bfloat16)
        in_maps.append({
            "xt": xtr,
            "wt": wtf if fwd else wtb,
            "rt": rtf if fwd else rtb,
            "bcw": bcwf if fwd else bcwb,
        })

    res = run_bass_kernel_spmd(nc, in_maps, core_ids=list(range(8)))

    TC0 = -(-T // (KL * MB))
    outp = np.empty((16, T, 2 * U), dtype=np.float32)
    for core in range(8):
        od = np.asarray(res.results[core]["out"]).astype(np.float32)
        TS0 = TC0 + PW + 1
        od = od.reshape(2, 128, MB, KL, BL, TS0)[..., PW + 1:PW + 1 + TC0]
        # [kp, p, b, j, b4, t'] -> [b4, (b j t'), kp*128+p]
        hb = od.transpose(4, 2, 3, 5, 0, 1).reshape(BL, -1, U)[:, :T]
        b0 = (core % 4) * BL
        if core < 4:
            outp[b0:b0 + BL, :, 0:U] = hb
        else:
            outp[b0:b0 + BL, :, U:2 * U] = hb
    return outp
